# revision 27
# baseline (speedup 1.0000x reference)
"""Trainium2 Bass kernel for nn_DetectionLoss (YOLO-style detection loss).

Pure data-parallel over batch: 8 cores x 4096 samples, 32 samples per
partition, dense work in 2 half-chunks of 16 samples (ND=1568 cells per
partition per half).

Layout: the host pre-casts predictions to f16 and transposes them to
channel-major per partition ([P, 8 ch, 3136 cells]) so every dense
operand is a packed 2-byte vector (DVE 2x/4x perf modes, half the HBM
bytes). Box inputs are marshalled into f16 planes (cx_sum, cy_sum, w, h,
valid, label+1, class_weight).

GPSIMD local_scatter processes indices in order -> duplicate cell
assignments resolve last-write-wins exactly like jax `.at[].set`
(verified on HW), so no duplicate-resolution pass is needed. 6 scatters
per half: label+1 and cw first (they unblock the mask-side dense ops),
then tx/ty/tw/th. The per-target stage computes the flat cell index
first so scatters start as early as possible.

Loss decomposition accumulated into per-partition partials columns:
  loss = 0.5*sum_all sp(po) + sum_m (0.5*sp(po) - po)
       + 2.5*(sum_m d^2 - sum_m relu(|d|-1)^2)
       + 2*(sum_m cw*logZ - sum_m cw*pc[lbl])
  total = loss / max(num_pos, 1)   (host)
"""
import sys

sys.path.insert(0, "/opt/trn_rl_repo")

import numpy as np

import concourse.bass as bass
import concourse.bacc as bacc
import concourse.tile as tile
from concourse import mybir
from concourse.bass_utils import run_bass_kernel_spmd

F32 = mybir.dt.float32
F16 = mybir.dt.float16
I32 = mybir.dt.int32
I16 = mybir.dt.int16
ALU = mybir.AluOpType
ACTF = mybir.ActivationFunctionType

G = 7
A = 2
C = 3
NCELL = G * G * A  # 98
ROW = 5 + C        # 8
M = 20
P = 128
N_CORES = 8
L_COORD, L_OBJ, L_NOOBJ, L_CLS = 5.0, 1.0, 0.5, 2.0

ANCHORS = np.array([[0.971, 1.7338], [3.4579, 5.1653]], dtype=np.float32)
CLASS_WEIGHTS = np.array([1.0, 4.9, 4.8], dtype=np.float32)

NCOL = 16  # partials columns per half (9 used; d2/rl2 split in two)


def _ap(t, offset_delta, dims):
    """Custom AP over tile/AP t: keep partition dim, replace free dims."""
    base = t[:] if not isinstance(t, bass.AP) else t
    return bass.AP(base.tensor, base.offset + offset_delta, [base.ap[0]] + dims)


def build_program(Q):
    """One-core SPMD program. B_core = 128*Q samples."""
    halves = 2
    Qc = Q // halves           # sample-groups per half per partition
    QM = Q * M                 # targets per partition (full width)
    QMh = Qc * M               # targets per partition per half
    ND = Qc * NCELL            # dense cells per partition per half
    assert ND * 32 < 2 ** 16   # local_scatter scratch limit
    nc = bacc.Bacc("TRN2", target_bir_lowering=False)

    predt = nc.dram_tensor("predt", [P, ROW * Q * NCELL], F16,
                           kind="ExternalInput")
    # planes: [cxs | cys | w | h | valid | cw], each [P, QM] f16
    bpl = nc.dram_tensor("bpl", [P, 6 * QM], F16, kind="ExternalInput")
    out_part = nc.dram_tensor("partials", [P, NCOL * halves], F32,
                              kind="ExternalOutput")

    a0w, a0h = float(ANCHORS[0, 0]), float(ANCHORS[0, 1])
    a1w, a1h = float(ANCHORS[1, 0]), float(ANCHORS[1, 1])
    lw0 = float(np.log(np.float32(a0w) + np.float32(1e-6)))
    lw1 = float(np.log(np.float32(a1w) + np.float32(1e-6)))
    lh0 = float(np.log(np.float32(a0h) + np.float32(1e-6)))
    lh1 = float(np.log(np.float32(a1h) + np.float32(1e-6)))
    c0_49 = (a0w * a0h + 1e-6) / 49.0
    c1_49 = (a1w * a1h + 1e-6) / 49.0
    w0, w1, w2 = [float(x) for x in CLASS_WEIGHTS]

    V = nc.vector
    S = nc.scalar
    GP = nc.gpsimd

    with tile.TileContext(nc) as tc:
        with (
            tc.tile_pool(name="const", bufs=1) as const,
            tc.tile_pool(name="io", bufs=1) as io,
            tc.tile_pool(name="tgt", bufs=1) as tgt,
            tc.tile_pool(name="dense", bufs=1) as dense,
        ):
            # ---------------- constants ----------------
            q98_i = const.tile([P, QM], I32, name="q98_i")
            GP.iota(q98_i[:], pattern=[[0, halves], [NCELL, Qc], [0, M]],
                    base=0, channel_multiplier=0)
            q98 = const.tile([P, QM], F16, name="q98")
            V.tensor_copy(q98[:], q98_i[:])
            partials = const.tile([P, NCOL * halves], F32, name="partials")
            V.memset(partials[:], 0.0)

            # ---------------- io tiles ----------------
            PL = io.tile([P, 6 * QM], F16, name="PL")
            # geometry planes first (gate the target stage), then valid/cw
            nc.sync.dma_start(out=PL[:, 0:4 * QM], in_=_ap(bpl, 0, [[1, 4 * QM]]))
            nc.sync.dma_start(out=PL[:, 4 * QM:6 * QM],
                              in_=_ap(bpl, 4 * QM, [[1, 2 * QM]]))
            PR = [io.tile([P, ROW * ND], F16, name=f"PR{h}")
                  for h in range(halves)]
            for h in range(halves):
                nc.sync.dma_start(
                    out=PR[h][:],
                    in_=_ap(predt, h * ND, [[Q * NCELL, ROW], [1, ND]]))
            TD4 = [io.tile([P, 4 * ND], F16, name=f"TD4_{h}")
                   for h in range(halves)]
            CWD = [io.tile([P, ND], F16, name=f"CWD{h}")
                   for h in range(halves)]
            TXY = io.tile([P, 2 * QM], F16, name="TXY")
            TWH = io.tile([P, 2 * QM], F16, name="TWH")
            IDX16 = io.tile([P, QM], I16, name="IDX16")

            cxys = _ap(PL, 0, [[1, 2 * QM]])
            wh = _ap(PL, 2 * QM, [[1, 2 * QM]])
            wv = _ap(PL, 2 * QM, [[1, QM]])
            hv = _ap(PL, 3 * QM, [[1, QM]])
            valid = _ap(PL, 4 * QM, [[1, QM]])

            # ------------- per-target stage (f16, x/y merged) -----------
            # Emission order puts the flat-index chain first so the
            # lbl/cw scatters (and the mask-side dense ops) start early.
            def t6(name):
                return tgt.tile([P, QM], F16, name=name)

            def t12(name):
                return tgt.tile([P, 2 * QM], F16, name=name)

            CXY7m = t12("CXY7m")
            V.tensor_scalar(CXY7m[:], cxys, 3.5, -0.5, op0=ALU.mult,
                            op1=ALU.add)
            GJI16 = tgt.tile([P, 2 * QM], I16, name="GJI16")
            V.tensor_copy(GJI16[:], CXY7m[:])   # round(x-0.5) == floor(x)
            GJIf = t12("GJIf")
            V.tensor_copy(GJIf[:], GJI16[:])

            # anchor argmax: best = (MW*c1/49 < W*(AR + c0/49 - I0))
            MW = t6("MW")
            V.tensor_scalar_min(MW[:], wv, a0w / 7.0)
            AR = t6("AR")
            V.tensor_tensor(AR[:], wv, hv, op=ALU.mult)
            I0t = t6("I0t")
            V.tensor_tensor(I0t[:], MW[:], hv, op=ALU.mult)
            V.tensor_scalar_add(AR[:], AR[:], c0_49)
            V.tensor_tensor(AR[:], AR[:], I0t[:], op=ALU.subtract)
            V.tensor_tensor(AR[:], wv, AR[:], op=ALU.mult)   # = L0
            MWC = t6("MWC")
            V.tensor_scalar_mul(MWC[:], MW[:], c1_49)
            BEST = t6("BEST")
            V.tensor_tensor(BEST[:], MWC[:], AR[:], op=ALU.is_lt)

            # flat cell index: ((gi*7+gj)*2 + best) + 98*(q%Qc); -1 invalid
            GI14 = t6("GI14")
            V.tensor_scalar_mul(GI14[:], _ap(GJIf, QM, [[1, QM]]), 14.0)
            V.scalar_tensor_tensor(GI14[:], _ap(GJIf, 0, [[1, QM]]), 2.0,
                                   GI14[:], op0=ALU.mult, op1=ALU.add)
            BQ = t6("BQ")
            V.tensor_tensor(BQ[:], BEST[:], q98[:], op=ALU.add)
            V.tensor_tensor(GI14[:], GI14[:], BQ[:], op=ALU.add)
            V.tensor_scalar_add(GI14[:], GI14[:], 1.0)
            V.tensor_tensor(GI14[:], GI14[:], valid, op=ALU.mult)
            V.tensor_scalar(IDX16[:], GI14[:], -1.0, None, op0=ALU.add)

            # tx/ty (after IDX16 so scatters aren't blocked on them)
            V.tensor_tensor(TXY[:], CXY7m[:], GJIf[:], op=ALU.subtract)
            V.tensor_scalar_add(TXY[:], TXY[:], 0.5)

            # tw/th = ln(7*max(w,1/700)) - ln(anchor+1e-6)
            MXWH = t12("MXWH")
            V.tensor_scalar_max(MXWH[:], wh, 1.0 / 700.0)
            LNWH = t12("LNWH")
            S.activation(LNWH[:], MXWH[:], ACTF.Ln, scale=7.0)
            AWHt = t12("AWHt")
            V.tensor_scalar(AWHt[:, 0:QM], BEST[:], lw1 - lw0, lw0,
                            op0=ALU.mult, op1=ALU.add)
            V.tensor_scalar(AWHt[:, QM:2 * QM], BEST[:], lh1 - lh0, lh0,
                            op0=ALU.mult, op1=ALU.add)
            V.tensor_tensor(TWH[:], LNWH[:], AWHt[:], op=ALU.subtract)

            # ---------------- dense per-half pipeline ----------------
            SPD = [dense.tile([P, ND], F16, name=f"SPD{h}")
                   for h in range(halves)]
            EZ = [dense.tile([P, C * ND], F16, name=f"EZ{h}")
                  for h in range(halves)]
            LWJ = dense.tile([P, ND], F16, name="LWJ")
            OBT = dense.tile([P, ND], F16, name="OBT")
            OBM = dense.tile([P, ND], F16, name="OBM")
            MKD = dense.tile([P, ND], F16, name="MKD")
            ZD = dense.tile([P, ND], F16, name="ZD")
            LZD = dense.tile([P, ND], F16, name="LZD")
            OH3 = dense.tile([P, C * ND], F16, name="OH3")
            DD4 = dense.tile([P, 4 * ND], F16, name="DD4")
            JNK4 = dense.tile([P, 4 * ND], F16, name="JNK4")

            EXPD = [dense.tile([P, ND], F16, name=f"EXPD{h}")
                    for h in range(halves)]
            # early ACT work, grouped by function to limit table loads:
            # all exps (both halves), then the softplus lns
            for h in range(halves):
                S.activation(EXPD[h][:], _ap(PR[h], 0, [[1, ND]]), ACTF.Exp)
                S.activation(EZ[h][:], _ap(PR[h], 5 * ND, [[1, C * ND]]),
                             ACTF.Exp)
            for h in range(halves):
                S.activation(SPD[h][:], EXPD[h][:], ACTF.Ln, bias=1.0,
                             accum_out=partials[:, h * NCOL:h * NCOL + 1])

            for h in range(halves):
                def col(i):
                    return partials[:, h * NCOL + i:h * NCOL + i + 1]

                tsl = slice(h * QMh, (h + 1) * QMh)
                tsly = slice(QM + h * QMh, QM + (h + 1) * QMh)

                # ---- scatters (Pool); cw first to unblock mask ops ----
                GP.local_scatter(
                    out_ap=CWD[h][:],
                    data_ap=_ap(PL, 5 * QM + h * QMh, [[1, QMh]]),
                    idxs_ap=IDX16[:, tsl],
                    channels=P, num_elems=ND, num_idxs=QMh)
                for k, dsl in enumerate([tsl, tsly]):
                    GP.local_scatter(
                        out_ap=TD4[h][:, k * ND:(k + 1) * ND],
                        data_ap=TXY[:, dsl], idxs_ap=IDX16[:, tsl],
                        channels=P, num_elems=ND, num_idxs=QMh)
                for k, dsl in enumerate([tsl, tsly]):
                    GP.local_scatter(
                        out_ap=TD4[h][:, (2 + k) * ND:(3 + k) * ND],
                        data_ap=TWH[:, dsl], idxs_ap=IDX16[:, tsl],
                        channels=P, num_elems=ND, num_idxs=QMh)

                po = _ap(PR[h], 0, [[1, ND]])
                pc3 = _ap(PR[h], 5 * ND, [[1, C * ND]])

                # ---- mask-side (needs only CWD scatter) ----
                V.tensor_scalar(MKD[:], CWD[h][:], 0.0, 0.0, op0=ALU.is_gt,
                                op1=ALU.add, accum_out=col(6))
                for c in range(C):
                    V.tensor_scalar(OH3[:, c * ND:(c + 1) * ND], CWD[h][:],
                                    [w0, w1, w2][c], [w0, w1, w2][c],
                                    op0=ALU.is_equal, op1=ALU.mult)
                # obj + CE pieces that only need EZ/OH3 (run during
                # scatters); ACT queue order: copies before the Ln
                V.tensor_scalar_mul(OBT[:], SPD[h][:], 0.5)
                V.tensor_tensor(OBT[:], OBT[:], po, op=ALU.subtract)
                V.tensor_tensor(OBM[:], OBT[:], MKD[:], op=ALU.mult)
                S.activation(OBM[:], OBM[:], ACTF.Copy, accum_out=col(1))
                V.tensor_tensor(ZD[:], _ap(EZ[h], 0, [[1, ND]]),
                                _ap(EZ[h], ND, [[1, ND]]), op=ALU.add)
                V.tensor_tensor(ZD[:], ZD[:], _ap(EZ[h], 2 * ND, [[1, ND]]),
                                op=ALU.add)
                S.activation(LZD[:], ZD[:], ACTF.Ln)
                V.tensor_tensor(OH3[:], OH3[:], pc3, op=ALU.mult)
                S.activation(OH3[:], OH3[:], ACTF.Copy, accum_out=col(5))

                # ---- smooth L1 (two 2-coord pieces as scatters land).
                # The lgt copy-accum is emitted between the pieces so it
                # fills ACT's wait for the second piece.
                for pi in range(2):
                    psl = slice(pi * 2 * ND, (pi + 1) * 2 * ND)
                    dd = DD4[:, psl]
                    for ci in range(2):
                        c0 = (2 * pi + ci) * ND
                        V.tensor_tensor(DD4[:, c0:c0 + ND],
                                        _ap(PR[h], ND + c0, [[1, ND]]),
                                        MKD[:], op=ALU.mult)
                    V.tensor_tensor(dd, dd, TD4[h][:, psl], op=ALU.subtract)
                    ddi = dd.bitcast(I16)
                    V.tensor_scalar(ddi, ddi, 0x7FFF, None,
                                    op0=ALU.bitwise_and)
                    if h == halves - 1 and pi == 1:
                        # final piece: d^2 on DVE so ACT isn't the lone tail
                        V.scalar_tensor_tensor(JNK4[:, psl], dd, 1.0, dd,
                                               op0=ALU.mult, op1=ALU.mult,
                                               accum_out=col(2 + 8 * pi))
                    else:
                        S.activation(JNK4[:, psl], dd, ACTF.Square,
                                     accum_out=col(2 + 8 * pi))
                    V.tensor_scalar(TD4[h][:, psl], dd, -1.0, 0.0,
                                    op0=ALU.add, op1=ALU.max)
                    S.activation(JNK4[:, psl], TD4[h][:, psl], ACTF.Square,
                                 accum_out=col(3 + 8 * pi))

                # lzw last (LZD ready long before; fills the square tail)
                V.scalar_tensor_tensor(LWJ[:], CWD[h][:], 1.0, LZD[:],
                                       op0=ALU.mult, op1=ALU.mult,
                                       accum_out=col(4))

            nc.sync.dma_start(out=out_part[:], in_=partials[:])

    nc.finalize()
    return nc


_CACHE = {}


def _get_program(Q):
    if Q not in _CACHE:
        _CACHE[Q] = build_program(Q)
    return _CACHE[Q]


def shard_inputs(predictions, target_boxes, target_labels, num_objs):
    B = predictions.shape[0]
    Bc = B // N_CORES
    Q = Bc // P
    QM = Q * M
    predt = predictions.astype(np.float16).reshape(N_CORES, P, Q, NCELL, ROW)
    predt = np.ascontiguousarray(predt.transpose(0, 1, 4, 2, 3)).reshape(
        N_CORES, P, ROW * Q * NCELL)
    tb = np.asarray(target_boxes, dtype=np.float32)
    x1, y1, x2, y2 = tb[..., 0], tb[..., 1], tb[..., 2], tb[..., 3]
    lbl = np.asarray(target_labels)
    nob = np.asarray(num_objs)
    cwt = np.asarray(CLASS_WEIGHTS, np.float32)
    bpl = np.empty((B, 6, M), np.float16)
    bpl[:, 0] = x1 + x2
    bpl[:, 1] = y1 + y2
    bpl[:, 2] = x2 - x1
    bpl[:, 3] = y2 - y1
    bpl[:, 4] = np.arange(M)[None, :] < nob[:, None]
    bpl[:, 5] = cwt[lbl]
    # [B, 6, M] -> per core [P, 6, Q*M]
    bpl = bpl.reshape(N_CORES, P, Q, 6, M).transpose(0, 1, 3, 2, 4)
    bpl = np.ascontiguousarray(bpl).reshape(N_CORES, P, 6 * QM)
    return [dict(predt=predt[i], bpl=bpl[i]) for i in range(N_CORES)]


def combine_partials(parts, halves=2):
    """parts: list of (P, NCOL*halves) arrays."""
    s = np.zeros(NCOL, np.float64)
    for p in parts:
        p = p.astype(np.float64)
        for h in range(halves):
            s += p[:, h * NCOL:(h + 1) * NCOL].sum(axis=0)
    sp_all, obj_t, lzw, lgt, npos = s[0], s[1], s[4], s[5], s[6]
    d2 = s[2] + s[10]
    rl2 = s[3] + s[11]
    loss_sum = (L_NOOBJ * sp_all + obj_t + L_COORD * 0.5 * (d2 - rl2)
                + L_CLS * (lzw - lgt))
    total = loss_sum / max(npos, 1.0)
    return np.float32(total)


LAST_EXEC_NS = [None]


def kernel(predictions, target_boxes, target_labels, num_objs,
           anchors=None, class_weights=None, **_):
    B = predictions.shape[0]
    Q = B // (N_CORES * P)
    nc = _get_program(Q)
    in_maps = shard_inputs(predictions, target_boxes, target_labels, num_objs)
    res = run_bass_kernel_spmd(nc, in_maps, core_ids=list(range(N_CORES)))
    LAST_EXEC_NS[0] = res.exec_time_ns
    return combine_partials([r["partials"] for r in res.results])


# revision 28
# speedup vs baseline: 1.0159x; 1.0159x over previous
"""Trainium2 Bass kernel for nn_DetectionLoss (YOLO-style detection loss).

Pure data-parallel over batch: 8 cores x 4096 samples, 32 samples per
partition, dense work in 2 half-chunks of 16 samples (ND=1568 cells per
partition per half).

Layout: the host pre-casts predictions to f16 and transposes them to
channel-major per partition ([P, 8 ch, 3136 cells]) so every dense
operand is a packed 2-byte vector (DVE 2x/4x perf modes, half the HBM
bytes). Box inputs are marshalled into f16 planes (cx_sum, cy_sum, w, h,
valid, label+1, class_weight).

GPSIMD local_scatter processes indices in order -> duplicate cell
assignments resolve last-write-wins exactly like jax `.at[].set`
(verified on HW), so no duplicate-resolution pass is needed. 6 scatters
per half: label+1 and cw first (they unblock the mask-side dense ops),
then tx/ty/tw/th. The per-target stage computes the flat cell index
first so scatters start as early as possible.

Loss decomposition accumulated into per-partition partials columns:
  loss = 0.5*sum_all sp(po) + sum_m (0.5*sp(po) - po)
       + 2.5*(sum_m d^2 - sum_m relu(|d|-1)^2)
       + 2*(sum_m cw*logZ - sum_m cw*pc[lbl])
  total = loss / max(num_pos, 1)   (host)
"""
import sys

sys.path.insert(0, "/opt/trn_rl_repo")

import numpy as np

import concourse.bass as bass
import concourse.bacc as bacc
import concourse.tile as tile
from concourse import mybir
from concourse.bass_utils import run_bass_kernel_spmd

F32 = mybir.dt.float32
F16 = mybir.dt.float16
I32 = mybir.dt.int32
I16 = mybir.dt.int16
ALU = mybir.AluOpType
ACTF = mybir.ActivationFunctionType

G = 7
A = 2
C = 3
NCELL = G * G * A  # 98
ROW = 5 + C        # 8
M = 20
P = 128
N_CORES = 8
L_COORD, L_OBJ, L_NOOBJ, L_CLS = 5.0, 1.0, 0.5, 2.0

ANCHORS = np.array([[0.971, 1.7338], [3.4579, 5.1653]], dtype=np.float32)
CLASS_WEIGHTS = np.array([1.0, 4.9, 4.8], dtype=np.float32)

NCOL = 16  # partials columns per half (9 used; d2/rl2 split in two)


def _ap(t, offset_delta, dims):
    """Custom AP over tile/AP t: keep partition dim, replace free dims."""
    base = t[:] if not isinstance(t, bass.AP) else t
    return bass.AP(base.tensor, base.offset + offset_delta, [base.ap[0]] + dims)


def build_program(Q):
    """One-core SPMD program. B_core = 128*Q samples."""
    halves = 2
    Qc = Q // halves           # sample-groups per half per partition
    QM = Q * M                 # targets per partition (full width)
    QMh = Qc * M               # targets per partition per half
    ND = Qc * NCELL            # dense cells per partition per half
    assert ND * 32 < 2 ** 16   # local_scatter scratch limit
    nc = bacc.Bacc("TRN2", target_bir_lowering=False)

    predt = nc.dram_tensor("predt", [P, ROW * Q * NCELL], F16,
                           kind="ExternalInput")
    # planes: [cxs | cys | w | h | valid | cw], each [P, QM] f16
    bpl = nc.dram_tensor("bpl", [P, 6 * QM], F16, kind="ExternalInput")
    out_part = nc.dram_tensor("partials", [P, NCOL * halves], F32,
                              kind="ExternalOutput")

    a0w, a0h = float(ANCHORS[0, 0]), float(ANCHORS[0, 1])
    a1w, a1h = float(ANCHORS[1, 0]), float(ANCHORS[1, 1])
    lw0 = float(np.log(np.float32(a0w) + np.float32(1e-6)))
    lw1 = float(np.log(np.float32(a1w) + np.float32(1e-6)))
    lh0 = float(np.log(np.float32(a0h) + np.float32(1e-6)))
    lh1 = float(np.log(np.float32(a1h) + np.float32(1e-6)))
    c0_49 = (a0w * a0h + 1e-6) / 49.0
    c1_49 = (a1w * a1h + 1e-6) / 49.0
    w0, w1, w2 = [float(x) for x in CLASS_WEIGHTS]

    V = nc.vector
    S = nc.scalar
    GP = nc.gpsimd

    with tile.TileContext(nc) as tc:
        with (
            tc.tile_pool(name="const", bufs=1) as const,
            tc.tile_pool(name="io", bufs=1) as io,
            tc.tile_pool(name="tgt", bufs=1) as tgt,
            tc.tile_pool(name="dense", bufs=1) as dense,
        ):
            # ---------------- constants ----------------
            q98_i = const.tile([P, QM], I32, name="q98_i")
            GP.iota(q98_i[:], pattern=[[0, halves], [NCELL, Qc], [0, M]],
                    base=0, channel_multiplier=0)
            q98 = const.tile([P, QM], F16, name="q98")
            V.tensor_copy(q98[:], q98_i[:])
            partials = const.tile([P, NCOL * halves], F32, name="partials")
            V.memset(partials[:], 0.0)

            # ---------------- io tiles ----------------
            PL = io.tile([P, 6 * QM], F16, name="PL")
            # geometry planes first (gate the target stage), then valid/cw
            nc.sync.dma_start(out=PL[:, 0:4 * QM], in_=_ap(bpl, 0, [[1, 4 * QM]]))
            nc.sync.dma_start(out=PL[:, 4 * QM:6 * QM],
                              in_=_ap(bpl, 4 * QM, [[1, 2 * QM]]))
            PR = [io.tile([P, ROW * ND], F16, name=f"PR{h}")
                  for h in range(halves)]
            for h in range(halves):
                nc.sync.dma_start(
                    out=PR[h][:],
                    in_=_ap(predt, h * ND, [[Q * NCELL, ROW], [1, ND]]))
            TD4 = [io.tile([P, 4 * ND], F16, name=f"TD4_{h}")
                   for h in range(halves)]
            CWD = [io.tile([P, ND], F16, name=f"CWD{h}")
                   for h in range(halves)]
            TXY = io.tile([P, 2 * QM], F16, name="TXY")
            TWH = io.tile([P, 2 * QM], F16, name="TWH")
            IDX16 = io.tile([P, QM], I16, name="IDX16")

            cxys = _ap(PL, 0, [[1, 2 * QM]])
            wh = _ap(PL, 2 * QM, [[1, 2 * QM]])
            wv = _ap(PL, 2 * QM, [[1, QM]])
            hv = _ap(PL, 3 * QM, [[1, QM]])
            valid = _ap(PL, 4 * QM, [[1, QM]])

            # ------------- per-target stage (f16, x/y merged) -----------
            # Emission order puts the flat-index chain first so the
            # lbl/cw scatters (and the mask-side dense ops) start early.
            def t6(name):
                return tgt.tile([P, QM], F16, name=name)

            def t12(name):
                return tgt.tile([P, 2 * QM], F16, name=name)

            CXY7m = t12("CXY7m")
            V.tensor_scalar(CXY7m[:], cxys, 3.5, -0.5, op0=ALU.mult,
                            op1=ALU.add)
            GJI16 = tgt.tile([P, 2 * QM], I16, name="GJI16")
            V.tensor_copy(GJI16[:], CXY7m[:])   # round(x-0.5) == floor(x)
            GJIf = t12("GJIf")
            V.tensor_copy(GJIf[:], GJI16[:])

            # anchor argmax: best = (MW*c1/49 < W*(AR + c0/49 - I0))
            MW = t6("MW")
            V.tensor_scalar_min(MW[:], wv, a0w / 7.0)
            AR = t6("AR")
            V.tensor_tensor(AR[:], wv, hv, op=ALU.mult)
            I0t = t6("I0t")
            V.tensor_tensor(I0t[:], MW[:], hv, op=ALU.mult)
            V.tensor_scalar_add(AR[:], AR[:], c0_49)
            V.tensor_tensor(AR[:], AR[:], I0t[:], op=ALU.subtract)
            V.tensor_tensor(AR[:], wv, AR[:], op=ALU.mult)   # = L0
            MWC = t6("MWC")
            V.tensor_scalar_mul(MWC[:], MW[:], c1_49)
            BEST = t6("BEST")
            V.tensor_tensor(BEST[:], MWC[:], AR[:], op=ALU.is_lt)

            # flat cell index: ((gi*7+gj)*2 + best) + 98*(q%Qc); -1 invalid
            GI14 = t6("GI14")
            V.tensor_scalar_mul(GI14[:], _ap(GJIf, QM, [[1, QM]]), 14.0)
            V.scalar_tensor_tensor(GI14[:], _ap(GJIf, 0, [[1, QM]]), 2.0,
                                   GI14[:], op0=ALU.mult, op1=ALU.add)
            BQ = t6("BQ")
            V.tensor_tensor(BQ[:], BEST[:], q98[:], op=ALU.add)
            V.tensor_tensor(GI14[:], GI14[:], BQ[:], op=ALU.add)
            V.tensor_scalar_add(GI14[:], GI14[:], 1.0)
            V.tensor_tensor(GI14[:], GI14[:], valid, op=ALU.mult)
            V.tensor_scalar(IDX16[:], GI14[:], -1.0, None, op0=ALU.add)

            # tx/ty (after IDX16 so scatters aren't blocked on them)
            V.tensor_tensor(TXY[:], CXY7m[:], GJIf[:], op=ALU.subtract)
            V.tensor_scalar_add(TXY[:], TXY[:], 0.5)

            # tw/th = ln(7*max(w,1/700)) - ln(anchor+1e-6)
            MXWH = t12("MXWH")
            V.tensor_scalar_max(MXWH[:], wh, 1.0 / 700.0)
            LNWH = t12("LNWH")
            S.activation(LNWH[:], MXWH[:], ACTF.Ln, scale=7.0)
            AWHt = t12("AWHt")
            V.tensor_scalar(AWHt[:, 0:QM], BEST[:], lw1 - lw0, lw0,
                            op0=ALU.mult, op1=ALU.add)
            V.tensor_scalar(AWHt[:, QM:2 * QM], BEST[:], lh1 - lh0, lh0,
                            op0=ALU.mult, op1=ALU.add)
            V.tensor_tensor(TWH[:], LNWH[:], AWHt[:], op=ALU.subtract)

            # ---------------- dense per-half pipeline ----------------
            SPD = [dense.tile([P, ND], F16, name=f"SPD{h}")
                   for h in range(halves)]
            EZ = [dense.tile([P, C * ND], F16, name=f"EZ{h}")
                  for h in range(halves)]
            LWJ = dense.tile([P, ND], F16, name="LWJ")
            OBT = dense.tile([P, ND], F16, name="OBT")
            OBM = dense.tile([P, ND], F16, name="OBM")
            MKD = dense.tile([P, ND], F16, name="MKD")
            ZD = dense.tile([P, ND], F16, name="ZD")
            LZD = dense.tile([P, ND], F16, name="LZD")
            OH3 = dense.tile([P, C * ND], F16, name="OH3")
            DD4 = dense.tile([P, 4 * ND], F16, name="DD4")
            JNK4 = dense.tile([P, 4 * ND], F16, name="JNK4")

            EXPD = [dense.tile([P, ND], F16, name=f"EXPD{h}")
                    for h in range(halves)]
            # early ACT work, grouped by function to limit table loads:
            # all exps (both halves), then the softplus lns
            for h in range(halves):
                S.activation(EXPD[h][:], _ap(PR[h], 0, [[1, ND]]), ACTF.Exp)
                S.activation(EZ[h][:], _ap(PR[h], 5 * ND, [[1, C * ND]]),
                             ACTF.Exp)
            for h in range(halves):
                S.activation(SPD[h][:], EXPD[h][:], ACTF.Ln, bias=1.0,
                             accum_out=partials[:, h * NCOL:h * NCOL + 1])

            for h in range(halves):
                def col(i):
                    return partials[:, h * NCOL + i:h * NCOL + i + 1]

                tsl = slice(h * QMh, (h + 1) * QMh)
                tsly = slice(QM + h * QMh, QM + (h + 1) * QMh)

                # ---- scatters (Pool); cw first to unblock mask ops ----
                GP.local_scatter(
                    out_ap=CWD[h][:],
                    data_ap=_ap(PL, 5 * QM + h * QMh, [[1, QMh]]),
                    idxs_ap=IDX16[:, tsl],
                    channels=P, num_elems=ND, num_idxs=QMh)
                for k, dsl in enumerate([tsl, tsly]):
                    GP.local_scatter(
                        out_ap=TD4[h][:, k * ND:(k + 1) * ND],
                        data_ap=TXY[:, dsl], idxs_ap=IDX16[:, tsl],
                        channels=P, num_elems=ND, num_idxs=QMh)
                for k, dsl in enumerate([tsl, tsly]):
                    GP.local_scatter(
                        out_ap=TD4[h][:, (2 + k) * ND:(3 + k) * ND],
                        data_ap=TWH[:, dsl], idxs_ap=IDX16[:, tsl],
                        channels=P, num_elems=ND, num_idxs=QMh)

                po = _ap(PR[h], 0, [[1, ND]])
                pc3 = _ap(PR[h], 5 * ND, [[1, C * ND]])

                # ---- mask-side (needs only CWD scatter) ----
                V.tensor_scalar(MKD[:], CWD[h][:], 0.0, 0.0, op0=ALU.is_gt,
                                op1=ALU.add, accum_out=col(6))
                for c in range(C):
                    V.tensor_scalar(OH3[:, c * ND:(c + 1) * ND], CWD[h][:],
                                    [w0, w1, w2][c], [w0, w1, w2][c],
                                    op0=ALU.is_equal, op1=ALU.mult)
                # obj + CE pieces that only need EZ/OH3 (run during
                # scatters); ACT queue order: copies before the Ln
                V.tensor_scalar_mul(OBT[:], SPD[h][:], 0.5)
                V.tensor_tensor(OBT[:], OBT[:], po, op=ALU.subtract)
                V.scalar_tensor_tensor(OBM[:], MKD[:], 1.0, OBT[:],
                                       op0=ALU.mult, op1=ALU.mult,
                                       accum_out=col(1))
                V.tensor_tensor(ZD[:], _ap(EZ[h], 0, [[1, ND]]),
                                _ap(EZ[h], ND, [[1, ND]]), op=ALU.add)
                V.tensor_tensor(ZD[:], ZD[:], _ap(EZ[h], 2 * ND, [[1, ND]]),
                                op=ALU.add)
                S.activation(LZD[:], ZD[:], ACTF.Ln)
                V.tensor_tensor(OH3[:], OH3[:], pc3, op=ALU.mult)
                S.activation(OH3[:], OH3[:], ACTF.Copy, accum_out=col(5))

                # ---- smooth L1 (two 2-coord pieces as scatters land).
                # The lgt copy-accum is emitted between the pieces so it
                # fills ACT's wait for the second piece.
                for pi in range(2):
                    psl = slice(pi * 2 * ND, (pi + 1) * 2 * ND)
                    dd = DD4[:, psl]
                    for ci in range(2):
                        c0 = (2 * pi + ci) * ND
                        V.tensor_tensor(DD4[:, c0:c0 + ND],
                                        _ap(PR[h], ND + c0, [[1, ND]]),
                                        MKD[:], op=ALU.mult)
                    V.tensor_tensor(dd, dd, TD4[h][:, psl], op=ALU.subtract)
                    ddi = dd.bitcast(I16)
                    V.tensor_scalar(ddi, ddi, 0x7FFF, None,
                                    op0=ALU.bitwise_and)
                    S.activation(JNK4[:, psl], dd, ACTF.Square,
                                 accum_out=col(2 + 8 * pi))
                    V.tensor_scalar(TD4[h][:, psl], dd, -1.0, 0.0,
                                    op0=ALU.add, op1=ALU.max)
                    S.activation(JNK4[:, psl], TD4[h][:, psl], ACTF.Square,
                                 accum_out=col(3 + 8 * pi))

                # lzw last (LZD ready long before; fills the square tail)
                V.scalar_tensor_tensor(LWJ[:], CWD[h][:], 1.0, LZD[:],
                                       op0=ALU.mult, op1=ALU.mult,
                                       accum_out=col(4))

            nc.sync.dma_start(out=out_part[:], in_=partials[:])

    nc.finalize()
    return nc


_CACHE = {}


def _get_program(Q):
    if Q not in _CACHE:
        _CACHE[Q] = build_program(Q)
    return _CACHE[Q]


def shard_inputs(predictions, target_boxes, target_labels, num_objs):
    B = predictions.shape[0]
    Bc = B // N_CORES
    Q = Bc // P
    QM = Q * M
    predt = predictions.astype(np.float16).reshape(N_CORES, P, Q, NCELL, ROW)
    predt = np.ascontiguousarray(predt.transpose(0, 1, 4, 2, 3)).reshape(
        N_CORES, P, ROW * Q * NCELL)
    tb = np.asarray(target_boxes, dtype=np.float32)
    x1, y1, x2, y2 = tb[..., 0], tb[..., 1], tb[..., 2], tb[..., 3]
    lbl = np.asarray(target_labels)
    nob = np.asarray(num_objs)
    cwt = np.asarray(CLASS_WEIGHTS, np.float32)
    bpl = np.empty((B, 6, M), np.float16)
    bpl[:, 0] = x1 + x2
    bpl[:, 1] = y1 + y2
    bpl[:, 2] = x2 - x1
    bpl[:, 3] = y2 - y1
    bpl[:, 4] = np.arange(M)[None, :] < nob[:, None]
    bpl[:, 5] = cwt[lbl]
    # [B, 6, M] -> per core [P, 6, Q*M]
    bpl = bpl.reshape(N_CORES, P, Q, 6, M).transpose(0, 1, 3, 2, 4)
    bpl = np.ascontiguousarray(bpl).reshape(N_CORES, P, 6 * QM)
    return [dict(predt=predt[i], bpl=bpl[i]) for i in range(N_CORES)]


def combine_partials(parts, halves=2):
    """parts: list of (P, NCOL*halves) arrays."""
    s = np.zeros(NCOL, np.float64)
    for p in parts:
        p = p.astype(np.float64)
        for h in range(halves):
            s += p[:, h * NCOL:(h + 1) * NCOL].sum(axis=0)
    sp_all, obj_t, lzw, lgt, npos = s[0], s[1], s[4], s[5], s[6]
    d2 = s[2] + s[10]
    rl2 = s[3] + s[11]
    loss_sum = (L_NOOBJ * sp_all + obj_t + L_COORD * 0.5 * (d2 - rl2)
                + L_CLS * (lzw - lgt))
    total = loss_sum / max(npos, 1.0)
    return np.float32(total)


LAST_EXEC_NS = [None]


def kernel(predictions, target_boxes, target_labels, num_objs,
           anchors=None, class_weights=None, **_):
    B = predictions.shape[0]
    Q = B // (N_CORES * P)
    nc = _get_program(Q)
    in_maps = shard_inputs(predictions, target_boxes, target_labels, num_objs)
    res = run_bass_kernel_spmd(nc, in_maps, core_ids=list(range(N_CORES)))
    LAST_EXEC_NS[0] = res.exec_time_ns
    return combine_partials([r["partials"] for r in res.results])
